# revision 11
# baseline (speedup 1.0000x reference)
"""CircleLossV2 on 8 Trainium2 NeuronCores (Bass/Tile).

Data-parallel: host normalizes/sorts/rotates; each core computes its 1024
rows x 8192 cols of the similarity matrix in fp16 matmuls, stages
v = s + 0.75 (PSUM->SBUF), squares on Pool, exp+rowsum on ACT with fixed
logsumexp shifts; same-class corrections from a 256-wide sorted-label
window.  Host epilogue does log/softplus/mean.

stats [128, 49] = [NS (32) | CR (9, col 8 = t0 wrap) | PS (8)].
"""

import sys

sys.path.insert(0, "/opt/trn_rl_repo")

import numpy as np
from ml_dtypes import bfloat16 as _bf16

import concourse.bass as bass
import concourse.bacc as bacc
import concourse.mybir as mybir
import concourse.tile as tile
from concourse.bass_utils import run_bass_kernel_spmd

F32 = mybir.dt.float32
F16 = mybir.dt.bfloat16
BF16 = mybir.dt.bfloat16
AF = mybir.ActivationFunctionType
OP = mybir.AluOpType

B = 8192
D = 128
NCORES = 8
RPC = B // NCORES  # rows per core
NT = RPC // 128  # row tiles per core (8)
WIN = 256  # pos window width
MHN = 140.0  # fixed LSE shift, negative logits
MHP = 100.0  # fixed LSE shift, positive logits
NE = 4  # exp instructions per tile (dense)

_PROG = None


def _register_const(nc, val, dtype=F32):
    t = nc.alloc_sbuf_tensor(f"uconst-{dtype.name}-{val}", [128, 1], dtype)
    nc.gpsimd.memset(t.ap(), val)
    nc.const_aps.aps[(dtype, val)] = t.ap()


def _build():
    nc = bacc.Bacc("TRN2", target_bir_lowering=False, debug=False, num_devices=NCORES)
    for v in (-MHN, -MHP):
        _register_const(nc, v)
    nc.all_engine_barrier()

    xt_in = nc.dram_tensor("xt", [D, B], F16, kind="ExternalInput")
    masks_in = nc.dram_tensor("masks", [NT, 128, WIN], F16, kind="ExternalInput")
    i128_in = nc.dram_tensor("i128", [128, 128], F16, kind="ExternalInput")
    n2i_in = nc.dram_tensor("n2i", [128, 128], F16, kind="ExternalInput")
    stats_out = nc.dram_tensor("stats", [128, 49], F32, kind="ExternalOutput")

    with tile.TileContext(nc) as tc:
        with (
            tc.tile_pool(name="cst", bufs=1) as cst,
            tc.tile_pool(name="ub", bufs=2) as ub,
            tc.tile_pool(name="wb", bufs=2) as wb,
            tc.tile_pool(name="psd", bufs=4, space="PSUM") as psd,
        ):
            # ---------------- constants / inputs ----------------
            eT = cst.tile([128, B], F16, tag="eT", name="eT")
            nc.sync.dma_start(eT[:], xt_in.ap())

            i128s = cst.tile([128, 128], F16, tag="i128s", name="i128s")
            nc.sync.dma_start(i128s[:], i128_in.ap())
            i128 = cst.tile([128, 128], F16, tag="i128", name="i128")
            nc.vector.tensor_copy(i128[:], i128s[:])

            n2is = cst.tile([128, 128], F16, tag="n2is", name="n2is")
            nc.sync.dma_start(n2is[:], n2i_in.ap())
            n2i = cst.tile([128, 128], F16, tag="n2i", name="n2i")
            nc.vector.tensor_copy(n2i[:], n2is[:])

            mts = []
            for t in range(NT):
                mt = cst.tile([128, WIN], F16, tag=f"mask{t}", name=f"mask{t}")
                nc.sync.dma_start(mt[:], masks_in.ap()[t, :, :])
                mts.append(mt)

            NS = cst.tile([128, NT * NE], F32, tag="NS", name="NS")
            CR = cst.tile([128, NT + 1], F32, tag="CR", name="CR")
            PS = cst.tile([128, NT], F32, tag="PS", name="PS")

            for t in range(NT):
                lhsT = eT[:, t * 128 : (t + 1) * 128]
                v = ub.tile([128, B], F16, tag="v", name=f"v{t}")
                u = ub.tile([128, B], F16, tag="u", name=f"u{t}")

                for c in range(8):
                    ps = psd.tile([128, 1024], F32, tag="psd", name=f"ps{t}_{c}")
                    for h in range(2):
                        has_diag = c == 0 and (t * 128) // 512 == h
                        nc.tensor.matmul(
                            ps[:, h * 512 : (h + 1) * 512],
                            lhsT,
                            eT[:, c * 1024 + h * 512 : c * 1024 + (h + 1) * 512],
                            start=True,
                            stop=not has_diag,
                        )
                        if has_diag:
                            nc.tensor.matmul(
                                ps[:, t * 128 : t * 128 + 128],
                                n2i[:],
                                i128[:],
                                start=False,
                                stop=True,
                                skip_group_check=True,
                            )
                    # stage v = s + 0.75 from PSUM to SBUF (DVE), square on
                    # Pool (SBUF-only engine)
                    vc = v[:, c * 1024 : (c + 1) * 1024]
                    nc.vector.tensor_scalar(vc, ps[:], 0.75, None, OP.add)
                    nc.gpsimd.tensor_tensor(
                        u[:, c * 1024 : (c + 1) * 1024], vc, vc, op=OP.mult
                    )

                W = B // NE
                for e in range(NE):
                    ee = ub.tile([128, W], BF16, tag="ee", name=f"ee{t}_{e}")
                    nc.scalar.activation(
                        ee[:], u[:, e * W : (e + 1) * W], AF.Exp, bias=-MHN,
                        scale=64.0, accum_out=NS[:, t * NE + e : t * NE + e + 1],
                    )

                # ---- window: CR (same-class neg corr) + PS (pos terms) ----
                um = wb.tile([128, WIN], F16, tag="um", name=f"um{t}")
                if t == 0:
                    nc.gpsimd.tensor_tensor(
                        um[:, 0:64], u[:, B - 64 : B], mts[0][:, 0:64], op=OP.mult
                    )
                    nc.gpsimd.tensor_tensor(
                        um[:, 64:WIN], u[:, 0:192], mts[0][:, 64:WIN], op=OP.mult
                    )
                else:
                    nc.gpsimd.tensor_tensor(
                        um[:], u[:, t * 128 - 64 : t * 128 + 192], mts[t][:],
                        op=OP.mult,
                    )
                cn = wb.tile([128, WIN], BF16, tag="cn", name=f"cn{t}")
                nc.scalar.activation(
                    cn[:], um[:], AF.Exp, bias=-MHN, scale=64.0,
                    accum_out=CR[:, t : t + 1],
                )

                vt = wb.tile([128, WIN], F16, tag="vt", name=f"vt{t}")
                if t == 0:
                    nc.vector.tensor_scalar(vt[:, 0:64], v[:, B - 64 : B], -1.5, None, OP.add)
                    nc.vector.tensor_scalar(vt[:, 64:WIN], v[:, 0:192], -1.5, None, OP.add)
                else:
                    nc.vector.tensor_scalar(
                        vt[:], v[:, t * 128 - 64 : t * 128 + 192], -1.5, None, OP.add
                    )
                uw = wb.tile([128, WIN], F16, tag="uw", name=f"uw{t}")
                nc.gpsimd.tensor_tensor(uw[:], vt[:], vt[:], op=OP.mult)
                vm = wb.tile([128, WIN], F16, tag="vm", name=f"vm{t}")
                nc.gpsimd.tensor_tensor(vm[:], uw[:], mts[t][:], op=OP.mult)
                epv = wb.tile([128, WIN], BF16, tag="ep", name=f"ep{t}")
                nc.scalar.activation(
                    epv[:], vm[:], AF.Exp, bias=-MHP, scale=64.0,
                    accum_out=PS[:, t : t + 1],
                )

            # ---------------- writeback ----------------
            NNS = NT * NE
            nc.sync.dma_start(stats_out.ap()[:, 0:NNS], NS[:])
            nc.sync.dma_start(stats_out.ap()[:, NNS : NNS + NT + 1], CR[:])
            nc.sync.dma_start(
                stats_out.ap()[:, NNS + NT + 1 : NNS + NT + 1 + NT], PS[:]
            )

    nc.compile()
    return nc


def _get_prog():
    global _PROG
    if _PROG is None:
        _PROG = _build()
    return _PROG


def _prepare_inputs(embeddings, labels):
    x = np.asarray(embeddings, dtype=np.float64)
    lab = np.asarray(labels)
    assert x.shape == (B, D) and lab.shape == (B,)

    e = x / np.linalg.norm(x, axis=1, keepdims=True)

    perm = np.argsort(lab, kind="stable")
    es = np.ascontiguousarray(e[perm])
    ls = lab[perm]

    _, inv_idx, counts = np.unique(ls, return_inverse=True, return_counts=True)
    cnt_row = counts[inv_idx]
    valid_sorted = (cnt_row >= 2) & (B - cnt_row >= 1)
    assert counts.max() <= 64, "window of 256 requires class size <= 64"

    ident = np.eye(128, dtype=_bf16)
    n2i = (-1.75 * np.eye(128)).astype(_bf16)

    in_maps = []
    for k in range(NCORES):
        sh = RPC * k
        er = np.roll(es, -sh, axis=0)
        lr = np.roll(ls, -sh)
        xt = np.ascontiguousarray(er.T).astype(_bf16)
        masks = np.zeros((NT, 128, WIN), dtype=_bf16)
        for t in range(NT):
            rows = lr[t * 128 : t * 128 + 128]
            wcols = np.arange(t * 128 - 64, t * 128 + 192) % B
            eq = rows[:, None] == lr[wcols][None, :]
            eq[np.arange(128), 64 + np.arange(128)] = False
            masks[t] = eq.astype(np.float32)
        in_maps.append(
            {
                "xt": xt,
                "masks": masks,
                "i128": ident,
                "n2i": n2i,
            }
        )
    return in_maps, valid_sorted


def _epilogue(results, valid_sorted):
    total = 0.0
    count = 0
    NNS = NT * NE
    for k in range(NCORES):
        st = np.asarray(results[k]["stats"], dtype=np.float64)
        ns = st[:, 0:NNS].reshape(128, NT, NE).sum(axis=2)  # [p, t]
        cr = st[:, NNS : NNS + NT + 1]
        ps_ = st[:, NNS + NT + 1 : NNS + NT + 1 + NT]
        cr_t = cr[:, :NT].copy()  # col NT (t0 wrap) unused in this variant

        neg = ns - cr_t
        p_idx = np.arange(128)[:, None]
        t_idx = np.arange(NT)[None, :]
        srow = (RPC * k + t_idx * 128 + p_idx) % B  # [p, t]
        vmask = valid_sorted[srow]

        with np.errstate(divide="ignore", invalid="ignore"):
            negterm = np.log(neg) + MHN
            posterm = np.log(ps_) + MHP
        xrow = negterm + posterm
        per_row = np.logaddexp(0.0, xrow)
        per_row = np.where(vmask, per_row, 0.0)
        total += per_row.sum()
        count += int(vmask.sum())
    return np.float32(total / max(count, 1))


def kernel(embeddings, labels, _trace=False):
    nc = _get_prog()
    in_maps, valid_sorted = _prepare_inputs(embeddings, labels)
    res = run_bass_kernel_spmd(
        nc, in_maps, core_ids=list(range(NCORES)), trace=_trace
    )
    loss = _epilogue(res.results, valid_sorted)
    if _trace:
        return loss, res
    return loss


# revision 15
# speedup vs baseline: 1.4115x; 1.4115x over previous
"""CircleLossV2 on 8 Trainium2 NeuronCores (Bass/Tile).

Data-parallel: host normalizes/sorts/rotates; each core computes its 1024
rows x 8192 cols of the similarity matrix in fp16 matmuls, stages
v = s + 0.75 (PSUM->SBUF), squares on Pool, exp+rowsum on ACT with fixed
logsumexp shifts; same-class corrections from a 256-wide sorted-label
window.  Host epilogue does log/softplus/mean.

stats [128, 49] = [NS (32) | CR (9, col 8 = t0 wrap) | PS (8)].
"""

import sys

sys.path.insert(0, "/opt/trn_rl_repo")

import numpy as np
from ml_dtypes import bfloat16 as _bf16

import concourse.bass as bass
import concourse.bacc as bacc
import concourse.mybir as mybir
import concourse.tile as tile
from concourse.bass_utils import run_bass_kernel_spmd

F32 = mybir.dt.float32
F16 = mybir.dt.bfloat16
BF16 = mybir.dt.bfloat16
AF = mybir.ActivationFunctionType
OP = mybir.AluOpType

B = 8192
D = 128
NCORES = 8
RPC = B // NCORES  # rows per core
NT = RPC // 128  # row tiles per core (8)
WIN = 256  # pos window width
MHN = 140.0  # fixed LSE shift, negative logits
MHP = 100.0  # fixed LSE shift, positive logits
NE = 2  # exp instructions per tile (dense)

_PROG = None


def _register_const(nc, val, dtype=F32):
    t = nc.alloc_sbuf_tensor(f"uconst-{dtype.name}-{val}", [128, 1], dtype)
    nc.gpsimd.memset(t.ap(), val)
    nc.const_aps.aps[(dtype, val)] = t.ap()


def _build():
    nc = bacc.Bacc("TRN2", target_bir_lowering=False, debug=False, num_devices=NCORES)
    for v in (-MHN, -MHP):
        _register_const(nc, v)
    nc.all_engine_barrier()

    xt_in = nc.dram_tensor("xt", [D, B], F16, kind="ExternalInput")
    masks_in = nc.dram_tensor("masks", [NT, 128, WIN], F16, kind="ExternalInput")
    i128_in = nc.dram_tensor("i128", [128, 128], F16, kind="ExternalInput")
    n2i_in = nc.dram_tensor("n2i", [128, 128], F16, kind="ExternalInput")
    stats_out = nc.dram_tensor("stats", [128, 49], F32, kind="ExternalOutput")

    with tile.TileContext(nc) as tc:
        with (
            tc.tile_pool(name="cst", bufs=1) as cst,
            tc.tile_pool(name="ub", bufs=2) as ub,
            tc.tile_pool(name="wb", bufs=2) as wb,
            tc.tile_pool(name="psd", bufs=4, space="PSUM") as psd,
        ):
            # ---------------- constants / inputs ----------------
            eT = cst.tile([128, B], F16, tag="eT", name="eT")
            nc.sync.dma_start(eT[:], xt_in.ap())

            i128s = cst.tile([128, 128], F16, tag="i128s", name="i128s")
            nc.sync.dma_start(i128s[:], i128_in.ap())
            i128 = cst.tile([128, 128], F16, tag="i128", name="i128")
            nc.vector.tensor_copy(i128[:], i128s[:])

            n2is = cst.tile([128, 128], F16, tag="n2is", name="n2is")
            nc.sync.dma_start(n2is[:], n2i_in.ap())
            n2i = cst.tile([128, 128], F16, tag="n2i", name="n2i")
            nc.vector.tensor_copy(n2i[:], n2is[:])

            mts = []
            for t in range(NT):
                mt = cst.tile([128, WIN], F16, tag=f"mask{t}", name=f"mask{t}")
                nc.sync.dma_start(mt[:], masks_in.ap()[t, :, :])
                mts.append(mt)

            NS = cst.tile([128, NT * NE], F32, tag="NS", name="NS")
            CR = cst.tile([128, NT + 1], F32, tag="CR", name="CR")
            PS = cst.tile([128, NT], F32, tag="PS", name="PS")

            for t in range(NT):
                lhsT = eT[:, t * 128 : (t + 1) * 128]
                v = ub.tile([128, B], F16, tag="v", name=f"v{t}")
                u = ub.tile([128, B], F16, tag="u", name=f"u{t}")

                for c in range(8):
                    ps = psd.tile([128, 1024], F32, tag="psd", name=f"ps{t}_{c}")
                    for h in range(2):
                        has_diag = c == 0 and (t * 128) // 512 == h
                        nc.tensor.matmul(
                            ps[:, h * 512 : (h + 1) * 512],
                            lhsT,
                            eT[:, c * 1024 + h * 512 : c * 1024 + (h + 1) * 512],
                            start=True,
                            stop=not has_diag,
                        )
                        if has_diag:
                            nc.tensor.matmul(
                                ps[:, t * 128 : t * 128 + 128],
                                n2i[:],
                                i128[:],
                                start=False,
                                stop=True,
                                skip_group_check=True,
                            )
                    # stage v = s + 0.75 from PSUM to SBUF (DVE), square on
                    # Pool (SBUF-only engine)
                    vc = v[:, c * 1024 : (c + 1) * 1024]
                    nc.vector.tensor_scalar(vc, ps[:], 0.75, None, OP.add)
                    sq_eng = nc.vector if c % 2 == 0 else nc.gpsimd
                    sq_eng.tensor_tensor(
                        u[:, c * 1024 : (c + 1) * 1024], vc, vc, op=OP.mult
                    )

                W = B // NE
                ees = []
                for e in range(NE):
                    ee = ub.tile([128, W], BF16, tag=f"ee{e}", name=f"ee{t}_{e}")
                    nc.scalar.activation(
                        ee[:], u[:, e * W : (e + 1) * W], AF.Exp, bias=-MHN,
                        scale=64.0, accum_out=NS[:, t * NE + e : t * NE + e + 1],
                    )
                    ees.append(ee)

                # ---- window: CR (same-class neg corr) + PS (pos terms) ----
                # CR re-exps the SAME u values the dense exp read (mask=1
                # keeps them bit-identical) so NS - CR cancels exactly
                um = wb.tile([128, WIN], F16, tag="um", name=f"um{t}")
                if t == 0:
                    nc.gpsimd.tensor_tensor(
                        um[:, 0:64], u[:, B - 64 : B], mts[0][:, 0:64], op=OP.mult
                    )
                    nc.gpsimd.tensor_tensor(
                        um[:, 64:WIN], u[:, 0:192], mts[0][:, 64:WIN], op=OP.mult
                    )
                else:
                    nc.gpsimd.tensor_tensor(
                        um[:], u[:, t * 128 - 64 : t * 128 + 192], mts[t][:],
                        op=OP.mult,
                    )
                cn = wb.tile([128, WIN], BF16, tag="cn", name=f"cn{t}")
                nc.scalar.activation(
                    cn[:], um[:], AF.Exp, bias=-MHN, scale=64.0,
                    accum_out=CR[:, t : t + 1],
                )

                vt = wb.tile([128, WIN], F16, tag="vt", name=f"vt{t}")
                if t == 0:
                    nc.vector.tensor_scalar(vt[:, 0:64], v[:, B - 64 : B], -1.5, None, OP.add)
                    nc.vector.tensor_scalar(vt[:, 64:WIN], v[:, 0:192], -1.5, None, OP.add)
                else:
                    nc.vector.tensor_scalar(
                        vt[:], v[:, t * 128 - 64 : t * 128 + 192], -1.5, None, OP.add
                    )
                uw = wb.tile([128, WIN], F16, tag="uw", name=f"uw{t}")
                nc.gpsimd.tensor_tensor(uw[:], vt[:], vt[:], op=OP.mult)
                vm = wb.tile([128, WIN], F16, tag="vm", name=f"vm{t}")
                nc.gpsimd.tensor_tensor(vm[:], uw[:], mts[t][:], op=OP.mult)
                epv = wb.tile([128, WIN], BF16, tag="ep", name=f"ep{t}")
                nc.scalar.activation(
                    epv[:], vm[:], AF.Exp, bias=-MHP, scale=64.0,
                    accum_out=PS[:, t : t + 1],
                )

            # ---------------- writeback ----------------
            NNS = NT * NE
            nc.sync.dma_start(stats_out.ap()[:, 0:NNS], NS[:])
            nc.sync.dma_start(stats_out.ap()[:, NNS : NNS + NT + 1], CR[:])
            nc.sync.dma_start(
                stats_out.ap()[:, NNS + NT + 1 : NNS + NT + 1 + NT], PS[:]
            )

    nc.compile()
    return nc


def _get_prog():
    global _PROG
    if _PROG is None:
        _PROG = _build()
    return _PROG


def _prepare_inputs(embeddings, labels):
    x = np.asarray(embeddings, dtype=np.float64)
    lab = np.asarray(labels)
    assert x.shape == (B, D) and lab.shape == (B,)

    e = x / np.linalg.norm(x, axis=1, keepdims=True)

    perm = np.argsort(lab, kind="stable")
    es = np.ascontiguousarray(e[perm])
    ls = lab[perm]

    _, inv_idx, counts = np.unique(ls, return_inverse=True, return_counts=True)
    cnt_row = counts[inv_idx]
    valid_sorted = (cnt_row >= 2) & (B - cnt_row >= 1)
    assert counts.max() <= 64, "window of 256 requires class size <= 64"

    ident = np.eye(128, dtype=_bf16)
    n2i = (-1.75 * np.eye(128)).astype(_bf16)

    in_maps = []
    for k in range(NCORES):
        sh = RPC * k
        er = np.roll(es, -sh, axis=0)
        lr = np.roll(ls, -sh)
        xt = np.ascontiguousarray(er.T).astype(_bf16)
        masks = np.zeros((NT, 128, WIN), dtype=_bf16)
        for t in range(NT):
            rows = lr[t * 128 : t * 128 + 128]
            wcols = np.arange(t * 128 - 64, t * 128 + 192) % B
            eq = rows[:, None] == lr[wcols][None, :]
            eq[np.arange(128), 64 + np.arange(128)] = False
            masks[t] = eq.astype(np.float32)
        in_maps.append(
            {
                "xt": xt,
                "masks": masks,
                "i128": ident,
                "n2i": n2i,
            }
        )
    return in_maps, valid_sorted


def _epilogue(results, valid_sorted):
    total = 0.0
    count = 0
    NNS = NT * NE
    for k in range(NCORES):
        st = np.asarray(results[k]["stats"], dtype=np.float64)
        ns = st[:, 0:NNS].reshape(128, NT, NE).sum(axis=2)  # [p, t]
        cr = st[:, NNS : NNS + NT + 1]
        ps_ = st[:, NNS + NT + 1 : NNS + NT + 1 + NT]
        cr_t = cr[:, :NT].copy()  # wrap merged on-device

        neg = ns - cr_t
        p_idx = np.arange(128)[:, None]
        t_idx = np.arange(NT)[None, :]
        srow = (RPC * k + t_idx * 128 + p_idx) % B  # [p, t]
        vmask = valid_sorted[srow]

        with np.errstate(divide="ignore", invalid="ignore"):
            negterm = np.log(neg) + MHN
            posterm = np.log(ps_) + MHP
        xrow = negterm + posterm
        per_row = np.logaddexp(0.0, xrow)
        per_row = np.where(vmask, per_row, 0.0)
        total += per_row.sum()
        count += int(vmask.sum())
    return np.float32(total / max(count, 1))


def kernel(embeddings, labels, _trace=False):
    nc = _get_prog()
    in_maps, valid_sorted = _prepare_inputs(embeddings, labels)
    res = run_bass_kernel_spmd(
        nc, in_maps, core_ids=list(range(NCORES)), trace=_trace
    )
    loss = _epilogue(res.results, valid_sorted)
    if _trace:
        return loss, res
    return loss
